# revision 27
# baseline (speedup 1.0000x reference)
"""TRN2 Bass kernel for nn_Block1_43542378447225 (fp16 rewrite).

Per sample on one NeuronCore (batch=2 -> cores 0/1 do real work):
  conv1 -> relu into padded a1p -> conv2 directly from strided a1p windows
  -> z2 -> Hopfield #1 in S^T layout -> Dm2 -> backward (w2b, Scomb) -> C
  -> blocked e_min (shifted candidate stack + min-reduce) -> eW gather
  -> mask -> masked patch forward (W1big) -> z2_masked -> Hopfield #2 -> out

All SBUF data fp16 (PE 4x faster than fp32, DVE 2x); PSUM fp32.
The C -> e_min -> mask comparison path stays bit-exact in fp16: cand/eW
matmuls are one-hot gathers, so every candidate equals an fp16-rounded C
entry exactly and the argmin survives `C16 <= eW`.

Hopfield is computed transpose-free: scores S^T[m, pq] = (K_chunk @ z) via
4 matmuls, exp on ACT, softmax denominators via an ones-column matmul.
Softmax 1/s is never applied to q on the spine: for Hopfield #1 the update
direction uses Dm2m' = (z2*(s/64) - q*m2), an s/64 column scale that
commutes through the backward and is cancelled once at C16 (rank-1
broadcast of 64/s); Hopfield #2 ships unnormalized q2 plus s2 and the
host divides.

Layout: pq = p*8+q (64 conv2 output positions), uv = u*10+v (100 composite
window offsets), kc = a*32 + c1 (hidden index; chunk t = conv2 kernel row,
a = conv2 kernel col).
"""
import numpy as np

import concourse.bass as bass
import concourse.bacc as bacc
import concourse.mybir as mybir
import concourse.tile as tile
from concourse.tile import add_dep_helper
from concourse.bass_utils import run_bass_kernel_spmd

F32 = mybir.dt.float32
F16 = mybir.dt.float16
AF = mybir.ActivationFunctionType
ALU = mybir.AluOpType

N_CORES = 8
BETA = 0.125  # 1/sqrt(64)

_CACHE = {}


# ---------------------------------------------------------------- host prep
def _build_scomb_w1big(w1):
    w1s = w1.sum(axis=1)
    Scomb = np.zeros((4, 32, 4, 100), np.float32)  # [a, c1, t, uv]
    W1big = np.zeros((100, 3, 4, 4, 32), np.float32)  # [uv, h, t, a, c1]
    for t in range(4):
        for a in range(4):
            for u in range(10):
                ki = u - 2 * t
                if not (0 <= ki < 4):
                    continue
                for v in range(10):
                    kj = v - 2 * a
                    if not (0 <= kj < 4):
                        continue
                    Scomb[a, :, t, u * 10 + v] = w1s[:, ki, kj]
                    W1big[u * 10 + v, :, t, a, :] = w1[:, :, ki, kj].T
    # partition index = a*32+c1 -> merge (a, c1); free = t*100+uv
    return Scomb.reshape(128, 400), W1big.reshape(100, 1536)


def _host_prep(w1, b1, w2, b2, K, Vw):
    # cv1 template [48, 289]: per-sample P1 (cols 0:256) filled later;
    # w1f | b1 shared. Small first DMA -> conv1 starts earliest.
    main = np.zeros((48, 289), np.float16)
    main[:, 256:288] = np.transpose(w1, (2, 3, 1, 0)).reshape(48, 32)
    main[0:32, 288:289] = b1[:, None]

    # conv2 weights with every (t, a) block based at partition 0:
    # cv2[c1, (t*4+a)*64 + o] = w2[o, c1, t, a]; b2 in col 1024
    cv2 = np.zeros((64, 1025), np.float16)
    cv2[0:32, 0:1024] = np.transpose(w2, (1, 2, 3, 0)).reshape(32, 1024)
    cv2[0:64, 1024:1025] = b2[:, None]

    hop = np.zeros((128, 832), np.float16)
    hop[0:64, 0:512] = K.T
    hop[0:64, 512:576] = Vw
    # KV chunks [128, 4, 64]: KV[m, e] = (K @ Vw)[m, e] / 64 (the 1/64 keeps
    # the s/64-scaled backward inside fp16 range; host divides it back out)
    KVh = (K @ Vw).astype(np.float32).reshape(4, 128, 64) / 64.0
    hop[:, 576:832] = np.transpose(KVh, (1, 0, 2)).reshape(128, 256)

    Scomb, W1big = _build_scomb_w1big(w1)
    PermF = np.zeros((100, 9, 16), np.float32)
    for k in range(9):
        dp, dq = k // 3 - 1, k % 3 - 1
        for im in range(4):
            u = 4 * dp + im + 3
            if not (0 <= u < 10):
                continue
            for jm in range(4):
                v = 4 * dq + jm + 3
                if not (0 <= v < 10):
                    continue
                PermF[u * 10 + v, k, im * 4 + jm] = 1.0
    CandM = np.zeros((100, 3, 128), np.float32)
    for k in range(9):
        cc, kk = divmod(k, 4)
        CandM[:, cc, kk * 32:kk * 32 + 16] = PermF[:, k, :]
    PermB = np.transpose(PermF, (2, 1, 0)).reshape(16, 900)

    # wB [128, 2196]: w2b (2.0 folded) | Scomb | CandM | PermB
    wB = np.zeros((128, 2196), np.float16)
    wB[0:64, 0:512] = 2.0 * np.transpose(w2, (0, 2, 3, 1)).reshape(64, 512)
    wB[:, 512:912] = Scomb
    wB[0:100, 912:1296] = CandM.reshape(100, 384)
    wB[0:16, 1296:2196] = PermB

    wC = np.zeros((128, 1792), np.float16)
    wC[0:100, 0:1536] = W1big
    wC[:, 1536:1792] = np.transpose(w2, (3, 1, 2, 0)).reshape(128, 256)
    return {"main": main, "cv2": cv2, "hop": hop, "wB": wB, "wC": wC}


def _sample_prep(x_s):
    xp1 = np.pad(x_s, ((0, 0), (1, 1), (1, 1)))
    xp3 = np.pad(x_s, ((0, 0), (3, 3), (3, 3)))
    P1 = np.zeros((4, 4, 3, 16, 16), np.float32)
    for kr in range(4):
        for ks in range(4):
            P1[kr, ks] = xp1[:, kr:kr + 32:2, ks:ks + 32:2][:, :16, :16]
    X = np.zeros((10, 10, 3, 8, 8), np.float32)
    for u in range(10):
        for v in range(10):
            X[u, v] = xp3[:, u:u + 32:4, v:v + 32:4][:, :8, :8]
    return (P1.reshape(48, 256).astype(np.float16),
            X.reshape(100, 192).astype(np.float16))


# ---------------------------------------------------------------- device build
def _hopfield(nc, sb, ps, z_sb, KT, KV, ones_col, tag):
    """z_sb [64(c), 64(pq)] fp16 -> (q_ps, s_ps): q_ps [64(c), 64(pq)] fp32
    PSUM = (K@Vw/64).T @ exp(beta S) UNNORMALIZED, s_ps [1, 64] fp32 PSUM =
    softmax denominators. Scores in S^T layout [m(4x128), pq]; no transposes,
    no normalization here (callers fold 1/s in downstream)."""
    ST = ps.tile([128, 256], F32, tag="S", bufs=2, name=f"ST{tag}")
    for t in range(4):
        nc.tensor.matmul(ST[:, t * 64:(t + 1) * 64],
                         KT[:, t * 128:(t + 1) * 128], z_sb,
                         start=True, stop=True)
    att = sb.tile([128, 256], F16, tag=f"att{tag}", name=f"att{tag}")
    nc.scalar.activation(out=att[:], in_=ST[:], func=AF.Exp,
                         bias=0.0, scale=BETA)
    qs_ps = ps.tile([65, 64], F32, tag="q64", bufs=2, name=f"qs{tag}")
    if tag == "2":
        # hf2: q first -- s2 is only DMA'd out, q gates the output copy
        for t in range(4):
            nc.tensor.matmul(qs_ps[0:64, :], KV[:, t, :],
                             att[:, t * 64:(t + 1) * 64],
                             start=(t == 0), stop=(t == 3))
        for t in range(4):
            nc.tensor.matmul(qs_ps[64:65, :], ones_col,
                             att[:, t * 64:(t + 1) * 64],
                             start=(t == 0), stop=(t == 3))
    else:
        for t in range(4):
            nc.tensor.matmul(qs_ps[64:65, :], ones_col,
                             att[:, t * 64:(t + 1) * 64],
                             start=(t == 0), stop=(t == 3))
        for t in range(4):
            nc.tensor.matmul(qs_ps[0:64, :], KV[:, t, :],
                             att[:, t * 64:(t + 1) * 64],
                             start=(t == 0), stop=(t == 3))
    return qs_ps


def _build_nc(debug=False):
    nc = bacc.Bacc("TRN2", target_bir_lowering=False, debug=False,
                   num_devices=N_CORES)
    d_main = nc.dram_tensor("main", [48, 289], F16, kind="ExternalInput")
    d_cv2 = nc.dram_tensor("cv2", [64, 1025], F16, kind="ExternalInput")
    d_hop = nc.dram_tensor("hop", [128, 832], F16, kind="ExternalInput")
    d_wB = nc.dram_tensor("wB", [128, 2196], F16, kind="ExternalInput")
    d_wC = nc.dram_tensor("wC", [128, 1792], F16, kind="ExternalInput")
    d_smpl = nc.dram_tensor("smpl", [100, 192], F16, kind="ExternalInput")
    out_t = nc.dram_tensor("out", [65, 64], F32, kind="ExternalOutput")

    with tile.TileContext(nc) as tc:
        with tc.tile_pool(name="sb", bufs=1) as sb, \
             tc.tile_pool(name="ps", bufs=1, space="PSUM") as ps:
            # ---- PE warm-up ASAP: pe_busy_start anchors the p-state ramp;
            # full speed arrives 3us after the first PE instruction.
            warm = sb.tile([2, 8], F16, tag="warm")
            nc.gpsimd.memset(warm[:], 0.0)
            for w_ in range(3):
                warm_ps = ps.tile([8, 8], F32, tag="q64", bufs=2,
                                  name=f"warm{w_}")
                nc.tensor.matmul(warm_ps[:], warm[0:2, :], warm[0:2, :],
                                 start=True, stop=True)

            # ---- input DMAs, ordered by first use (HWDGE serializes)
            main = sb.tile([48, 289], F16, tag="main")
            nc.sync.dma_start(out=main[:], in_=d_main[:])
            cv2 = sb.tile([64, 1025], F16, tag="cv2")
            nc.scalar.dma_start(out=cv2[:], in_=d_cv2[:])
            hop = sb.tile([128, 832], F16, tag="hop")
            nc.sync.dma_start(out=hop[:], in_=d_hop[:])
            wB = sb.tile([128, 2196], F16, tag="wB")
            nc.scalar.dma_start(out=wB[:], in_=d_wB[:])
            smpl = sb.tile([100, 192], F16, tag="smpl")
            nc.sync.dma_start(out=smpl[:], in_=d_smpl[:])
            wC = sb.tile([128, 1792], F16, tag="wC")
            nc.scalar.dma_start(out=wC[:], in_=d_wC[:])

            P1 = main[0:48, 0:256]
            w1f = main[0:48, 256:288]
            b1_16 = main[0:32, 288:289]
            b2_16 = cv2[0:64, 1024:1025]
            w2ta = cv2[0:32, 0:1024].rearrange("c (i o) -> c i o", i=16)
            w2fT = wC[:, 1536:1792].rearrange("k (t c) -> k t c", t=4)
            KT = hop[0:64, 0:512]
            KV = hop[:, 576:832].rearrange("k (t c) -> k t c", t=4)
            w2b = wB[0:64, 0:512]
            Scomb = wB[:, 512:912].rearrange("k (t u) -> k t u", t=4)
            CandM = wB[0:100, 912:1296].rearrange("u (c k) -> u c k", c=3)
            PermB = wB[0:16, 1296:2196]
            W1big = wC[0:100, 0:1536].rearrange("u (h t k) -> u h t k",
                                                h=3, t=4)
            X = smpl[:].rearrange("u (h q) -> u h q", h=3)

            # ---- Pool: constants + zero-fills, all off the critical path
            ones_col = sb.tile([128, 1], F16, tag="ones_col")
            nc.gpsimd.memset(ones_col[:], 1.0)
            ones_row = sb.tile([1, 100], F16, tag="ones_row")
            nc.gpsimd.memset(ones_row[:], 1.0)
            a1p = sb.tile([32, 18, 18], F16, tag="a1p")
            nc.gpsimd.memset(a1p[:], 0.0)
            cstk = sb.tile([16, 8, 8, 9], F16, tag="cstk")
            nc.gpsimd.memset(cstk[:], 0.0)
            eB = sb.tile([16, 12, 8], F16, tag="eB")
            nc.gpsimd.memset(eB[:], 0.0)

            # ---- biases to fp32 (DVE tensor_scalar needs fp32 scalar APs)
            b1c = sb.tile([32, 1], F32, tag="b1c")
            nc.vector.tensor_copy(out=b1c[:], in_=b1_16)

            # ---- conv1 + relu into padded a1p [32, 18, 18]
            a1_ps = ps.tile([32, 256], F32, tag="a1", bufs=1)
            nc.tensor.matmul(a1_ps[:], w1f, P1, start=True, stop=True)
            nc.vector.tensor_scalar(
                out=a1p[:, 1:17, 1:17],
                in0=a1_ps[:].rearrange("c (p q) -> c p q", p=16),
                scalar1=b1c[:], scalar2=0.0, op0=ALU.add, op1=ALU.max)

            # ---- conv2 + relu directly from strided a1p windows:
            # rhs(t,a)[c1, p, q] = a1p[c1, 2p+t, 2q+a]
            a1p_ap = a1p[:]
            z2_ps = ps.tile([64, 64], F32, tag="q64", bufs=2)
            i = 0
            for t in range(4):
                for a in range(4):
                    rhs = bass.AP(
                        tensor=a1p_ap.tensor,
                        offset=a1p_ap.offset + t * 18 + a,
                        ap=[[324, 32], [36, 8], [2, 8]])
                    nc.tensor.matmul(
                        z2_ps[:], w2ta[:, t * 4 + a, :], rhs,
                        start=(i == 0), stop=(i == 15))
                    i += 1
            z2 = sb.tile([64, 64], F16, tag="z2")
            nc.scalar.activation(out=z2[:], in_=z2_ps[:], func=AF.Relu,
                                 bias=b2_16, scale=1.0)

            # ---- relu-derivative masks, off the critical path:
            # M1W[a*32+c1, t, pq] = (a1p[c1, 2p+t, 2q+a] != 0)
            M1W = sb.tile([128, 4, 64], F16, tag="M1W")
            for a in range(4):
                src = bass.AP(
                    tensor=a1p_ap.tensor,
                    offset=a1p_ap.offset + a,
                    ap=[[324, 32], [18, 4], [36, 8], [2, 8]])
                dst = M1W[a * 32:(a + 1) * 32, :, :].rearrange(
                    "k t (p q) -> k t p q", p=8)
                nc.vector.tensor_scalar(out=dst, in0=src, scalar1=0.0,
                                        scalar2=None, op0=ALU.not_equal)
            m2 = sb.tile([64, 64], F16, tag="m2")
            nc.vector.tensor_scalar(out=m2[:], in0=z2[:], scalar1=0.0,
                                    scalar2=None, op0=ALU.not_equal)

            # ---- Hopfield #1. Instead of normalizing q (1/s on the free
            # axis is awkward), scale z2 by s/64: Dm2m' = (z2*(s/64) - q)*m2
            # = (s/64)*Dm2m_true, a column scale that commutes through the
            # whole backward; 1/(s/64) is applied once at C16, off-chain.
            qs1 = _hopfield(nc, sb, ps, z2[:], KT, KV, ones_col[:], "1")
            s1row = sb.tile([1, 64], F16, tag="s1row")
            nc.vector.tensor_scalar_mul(s1row[:], qs1[64:65, :], 1.0 / 64.0)
            # qm2 = q*m2 runs while the s-broadcast round-trips through PE
            qm2 = sb.tile([64, 64], F16, tag="qm2")
            nc.vector.tensor_tensor(out=qm2[:], in0=qs1[0:64, :], in1=m2[:],
                                    op=ALU.mult)
            sb1_ps = ps.tile([64, 64], F32, tag="q64", bufs=2, name="sb1")
            nc.tensor.matmul(sb1_ps[:], ones_row[0:1, 0:64], s1row[:],
                             start=True, stop=True)
            z2s = sb.tile([64, 64], F16, tag="z2s")
            nc.vector.tensor_tensor(out=z2s[:], in0=z2[:], in1=sb1_ps[:],
                                    op=ALU.mult)
            # (z2*sb1 - q)*m2 == z2s - q*m2 because z2*m2 == z2
            Dm2m = sb.tile([64, 64], F16, tag="Dm2m")
            dm2m_inst = nc.vector.tensor_tensor(out=Dm2m[:], in0=z2s[:],
                                                in1=qm2[:], op=ALU.subtract)
            # off-chain (issued after Dm2m so they don't sit on the spine):
            # recB100[uv, pq] = 64/s[pq] for the C un-scaling
            r1row = sb.tile([1, 64], F16, tag="r1row")
            with nc.allow_low_precision(reason="softmax 1/sum in fp16"):
                recip_inst = nc.vector.reciprocal(r1row[:], s1row[:])
            add_dep_helper(dm2m_inst.ins, recip_inst.ins, sync=False,
                           reason="recip only feeds C16; keep Dm2m spine hot")
            rb100_ps = ps.tile([100, 64], F32, tag="g128", bufs=3,
                               name="rb100")
            nc.tensor.matmul(rb100_ps[:], ones_row[:], r1row[:],
                             start=True, stop=True)
            recB100 = sb.tile([100, 64], F16, tag="recB100")
            nc.scalar.copy(out=recB100[:], in_=rb100_ps[:])

            # ---- backward: g1m = (w2b^T @ Dm2m) * M1W, all 4 chunks in one
            # PSUM tile + one DVE multiply
            g1_ps = ps.tile([128, 256], F32, tag="S", bufs=2)
            for t in range(4):
                nc.tensor.matmul(g1_ps[:, t * 64:(t + 1) * 64],
                                 w2b[:, t * 128:(t + 1) * 128], Dm2m[:],
                                 start=True, stop=True)
            g1m = sb.tile([128, 4, 64], F16, tag="g1m")
            nc.vector.tensor_tensor(
                out=g1m[:].rearrange("k t u -> k (t u)"), in0=g1_ps[:],
                in1=M1W[:].rearrange("k t u -> k (t u)"), op=ALU.mult)

            # ---- C [100, 64] = sum_t Scomb_t^T @ g1m_t, then fp16 snapshot
            C_ps = ps.tile([100, 64], F32, tag="a1", bufs=1)
            for t in range(4):
                nc.tensor.matmul(C_ps[:], Scomb[:, t, :], g1m[:, t, :],
                                 start=(t == 0), stop=(t == 3))
            C16 = sb.tile([100, 64], F16, tag="C16")
            nc.vector.tensor_tensor(out=C16[:], in0=C_ps[:], in1=recB100[:],
                                    op=ALU.mult)

            # ---- blocked e_min: 3 candidate matmuls, shifted stack, min
            cand = [None] * 3
            for cc in range(3):
                cand[cc] = ps.tile([128, 8, 8], F32, tag="g128", bufs=3,
                                   name=f"cand{cc}")
                nc.tensor.matmul(
                    cand[cc][:].rearrange("k p q -> k (p q)"),
                    CandM[:, cc, :], C16[:], start=True, stop=True)
            for j, k in enumerate([0, 1, 2, 3, 5, 6, 7, 8]):
                cc, kk = divmod(k, 4)
                dp, dq = k // 3 - 1, k % 3 - 1
                i4lo, i4hi = max(0, dp), min(8, 8 + dp)
                j4lo, j4hi = max(0, dq), min(8, 8 + dq)
                srcap = cand[cc][kk * 32:kk * 32 + 16,
                                 i4lo - dp:i4hi - dp,
                                 j4lo - dq:j4hi - dq, None]
                dstap = cstk[:, i4lo:i4hi, j4lo:j4hi, j:j + 1]
                if j % 2:
                    nc.scalar.copy(out=dstap, in_=srcap)
                else:
                    nc.vector.tensor_copy(out=dstap, in_=srcap)
            nc.vector.tensor_copy(out=cstk[:, :, :, 8:9],
                                  in_=cand[1][0:16, :, :, None])
            nc.vector.tensor_reduce(out=eB[:, 2:10, :], in_=cstk[:],
                                    axis=mybir.AxisListType.X, op=ALU.min)

            # ---- eW gather (one-hot PermB) + mask
            eBf = eB[:].rearrange("a b c -> a (b c)")
            eW_ps = ps.tile([100, 64], F32, tag="a1", bufs=1)
            for k in range(9):
                dp, dq = k // 3 - 1, k % 3 - 1
                off = 16 + 8 * dp + dq
                nc.tensor.matmul(eW_ps[:], PermB[:, k * 100:(k + 1) * 100],
                                 eBf[:, off:off + 64],
                                 start=(k == 0), stop=(k == 8))
            maskw = sb.tile([100, 64], F16, tag="maskw")
            nc.vector.tensor_tensor(out=maskw[:], in0=C16[:], in1=eW_ps[:],
                                    op=ALU.is_le)

            # ---- masked forward: Xm = X * maskw (broadcast over h)
            Xm = sb.tile([100, 3, 64], F16, tag="Xm")
            mask_b = bass.AP(tensor=maskw[:].tensor, offset=maskw[:].offset,
                             ap=[[64, 100], [0, 3], [1, 64]])
            nc.vector.tensor_tensor(out=Xm[:], in0=X, in1=mask_b, op=ALU.mult)
            u1_ps = ps.tile([128, 256], F32, tag="S", bufs=2)
            for t in range(4):
                for h in range(3):
                    nc.tensor.matmul(u1_ps[:, t * 64:(t + 1) * 64],
                                     W1big[:, h, t, :], Xm[:, h, :],
                                     start=(h == 0), stop=(h == 2))
            u1m = sb.tile([128, 4, 64], F16, tag="u1m")
            nc.vector.tensor_tensor(
                out=u1m[:].rearrange("k t u -> k (t u)"), in0=u1_ps[:],
                in1=M1W[:].rearrange("k t u -> k (t u)"), op=ALU.mult)
            zm_ps = ps.tile([64, 64], F32, tag="q64", bufs=2)
            for t in range(4):
                nc.tensor.matmul(zm_ps[:], w2fT[:, t, :], u1m[:, t, :],
                                 start=(t == 0), stop=(t == 3))
            z2m = sb.tile([64, 64], F16, tag="z2m")
            nc.vector.tensor_tensor(out=z2m[:], in0=zm_ps[:], in1=m2[:],
                                    op=ALU.mult)

            # ---- Hopfield #2 -> ship q2/64 (rows 0:64) and s2 (row 64);
            # the host computes out = 64*q2s/s2.
            qs2 = _hopfield(nc, sb, ps, z2m[:], KT, KV, ones_col[:], "2")
            out_sb = sb.tile([65, 64], F32, tag="out_sb")
            nc.vector.tensor_copy(out=out_sb[:], in_=qs2[:])
            nc.sync.dma_start(out=out_t[:], in_=out_sb[:])
    nc.compile()
    return nc


def _get_nc(debug=False):
    key = ("nc", debug)
    if key not in _CACHE:
        _CACHE[key] = _build_nc(debug)
    return _CACHE[key]


# ---------------------------------------------------------------- entry point
def kernel(x, w1, b1, w2, b2, K, Vw, _debug=False):
    x = np.asarray(x, np.float32)
    shared = _host_prep(np.asarray(w1, np.float32), np.asarray(b1, np.float32),
                        np.asarray(w2, np.float32), np.asarray(b2, np.float32),
                        np.asarray(K, np.float32), np.asarray(Vw, np.float32))
    bsz = x.shape[0]
    nc = _get_nc(False)
    smpls = [_sample_prep(x[b]) for b in range(bsz)]
    in_maps = []
    for core in range(N_CORES):
        P1b, Xb = smpls[core] if core < bsz else smpls[0]
        mainb = shared["main"].copy()
        mainb[0:48, 0:256] = P1b
        m = {"main": mainb, "cv2": shared["cv2"], "hop": shared["hop"],
             "wB": shared["wB"], "wC": shared["wC"], "smpl": Xb}
        in_maps.append(m)
    res = run_bass_kernel_spmd(nc, in_maps, core_ids=list(range(N_CORES)))
    outs = []
    for b in range(bsz):
        r = np.asarray(res.results[b]["out"], np.float32)
        outs.append((64.0 * r[0:64] / r[64:65]).reshape(64, 8, 8))
    out = np.stack(outs).astype(np.float32)
    if _debug:
        return out, res
    return out


# revision 28
# speedup vs baseline: 1.0053x; 1.0053x over previous
"""TRN2 Bass kernel for nn_Block1_43542378447225 (fp16 rewrite).

Per sample on one NeuronCore (batch=2 -> cores 0/1 do real work):
  conv1 -> relu into padded a1p -> conv2 directly from strided a1p windows
  -> z2 -> Hopfield #1 in S^T layout -> Dm2 -> backward (w2b, Scomb) -> C
  -> blocked e_min (shifted candidate stack + min-reduce) -> eW gather
  -> mask -> masked patch forward (W1big) -> z2_masked -> Hopfield #2 -> out

All SBUF data fp16 (PE 4x faster than fp32, DVE 2x); PSUM fp32.
The C -> e_min -> mask comparison path stays bit-exact in fp16: cand/eW
matmuls are one-hot gathers, so every candidate equals an fp16-rounded C
entry exactly and the argmin survives `C16 <= eW`.

Hopfield is computed transpose-free: scores S^T[m, pq] = (K_chunk @ z) via
4 matmuls, exp on ACT, softmax denominators via an ones-column matmul.
Softmax 1/s is never applied to q on the spine: for Hopfield #1 the update
direction uses Dm2m' = (z2*(s/64) - q*m2), an s/64 column scale that
commutes through the backward and is cancelled once at C16 (rank-1
broadcast of 64/s); Hopfield #2 ships unnormalized q2 plus s2 and the
host divides.

Layout: pq = p*8+q (64 conv2 output positions), uv = u*10+v (100 composite
window offsets), kc = a*32 + c1 (hidden index; chunk t = conv2 kernel row,
a = conv2 kernel col).
"""
import numpy as np

import concourse.bass as bass
import concourse.bacc as bacc
import concourse.mybir as mybir
import concourse.tile as tile
from concourse.tile import add_dep_helper
from concourse.bass_utils import run_bass_kernel_spmd

F32 = mybir.dt.float32
F16 = mybir.dt.float16
AF = mybir.ActivationFunctionType
ALU = mybir.AluOpType

N_CORES = 8
BETA = 0.125  # 1/sqrt(64)

_CACHE = {}


# ---------------------------------------------------------------- host prep
def _build_scomb_w1big(w1):
    w1s = w1.sum(axis=1)
    Scomb = np.zeros((4, 32, 4, 100), np.float32)  # [a, c1, t, uv]
    W1big = np.zeros((100, 3, 4, 4, 32), np.float32)  # [uv, h, t, a, c1]
    for t in range(4):
        for a in range(4):
            for u in range(10):
                ki = u - 2 * t
                if not (0 <= ki < 4):
                    continue
                for v in range(10):
                    kj = v - 2 * a
                    if not (0 <= kj < 4):
                        continue
                    Scomb[a, :, t, u * 10 + v] = w1s[:, ki, kj]
                    W1big[u * 10 + v, :, t, a, :] = w1[:, :, ki, kj].T
    # partition index = a*32+c1 -> merge (a, c1); free = t*100+uv
    return Scomb.reshape(128, 400), W1big.reshape(100, 1536)


def _host_prep(w1, b1, w2, b2, K, Vw):
    # cv1 template [48, 289]: per-sample P1 (cols 0:256) filled later;
    # w1f | b1 shared. Small first DMA -> conv1 starts earliest.
    main = np.zeros((48, 289), np.float16)
    main[:, 256:288] = np.transpose(w1, (2, 3, 1, 0)).reshape(48, 32)
    main[0:32, 288:289] = b1[:, None]

    # conv2 weights with every (t, a) block based at partition 0:
    # cv2[c1, (t*4+a)*64 + o] = w2[o, c1, t, a]; b2 in col 1024
    cv2 = np.zeros((64, 1025), np.float16)
    cv2[0:32, 0:1024] = np.transpose(w2, (1, 2, 3, 0)).reshape(32, 1024)
    cv2[0:64, 1024:1025] = b2[:, None]

    hop = np.zeros((128, 832), np.float16)
    hop[0:64, 0:512] = K.T
    hop[0:64, 512:576] = Vw
    # KV chunks [128, 4, 64]: KV[m, e] = (K @ Vw)[m, e] / 64 (the 1/64 keeps
    # the s/64-scaled backward inside fp16 range; host divides it back out)
    KVh = (K @ Vw).astype(np.float32).reshape(4, 128, 64) / 64.0
    hop[:, 576:832] = np.transpose(KVh, (1, 0, 2)).reshape(128, 256)

    Scomb, W1big = _build_scomb_w1big(w1)
    PermF = np.zeros((100, 9, 16), np.float32)
    for k in range(9):
        dp, dq = k // 3 - 1, k % 3 - 1
        for im in range(4):
            u = 4 * dp + im + 3
            if not (0 <= u < 10):
                continue
            for jm in range(4):
                v = 4 * dq + jm + 3
                if not (0 <= v < 10):
                    continue
                PermF[u * 10 + v, k, im * 4 + jm] = 1.0
    CandM = np.zeros((100, 3, 128), np.float32)
    for k in range(9):
        cc, kk = divmod(k, 4)
        CandM[:, cc, kk * 32:kk * 32 + 16] = PermF[:, k, :]
    PermB = np.transpose(PermF, (2, 1, 0)).reshape(16, 900)

    # wB [128, 2196]: w2b (2.0 folded) | Scomb | CandM | PermB
    wB = np.zeros((128, 2196), np.float16)
    wB[0:64, 0:512] = 2.0 * np.transpose(w2, (0, 2, 3, 1)).reshape(64, 512)
    wB[:, 512:912] = Scomb
    wB[0:100, 912:1296] = CandM.reshape(100, 384)
    wB[0:16, 1296:2196] = PermB

    wC = np.zeros((128, 1792), np.float16)
    wC[0:100, 0:1536] = W1big
    wC[:, 1536:1792] = np.transpose(w2, (3, 1, 2, 0)).reshape(128, 256)
    return {"main": main, "cv2": cv2, "hop": hop, "wB": wB, "wC": wC}


def _sample_prep(x_s):
    xp1 = np.pad(x_s, ((0, 0), (1, 1), (1, 1)))
    xp3 = np.pad(x_s, ((0, 0), (3, 3), (3, 3)))
    P1 = np.zeros((4, 4, 3, 16, 16), np.float32)
    for kr in range(4):
        for ks in range(4):
            P1[kr, ks] = xp1[:, kr:kr + 32:2, ks:ks + 32:2][:, :16, :16]
    X = np.zeros((10, 10, 3, 8, 8), np.float32)
    for u in range(10):
        for v in range(10):
            X[u, v] = xp3[:, u:u + 32:4, v:v + 32:4][:, :8, :8]
    return (P1.reshape(48, 256).astype(np.float16),
            X.reshape(100, 192).astype(np.float16))


# ---------------------------------------------------------------- device build
def _hopfield(nc, sb, ps, z_sb, KT, KV, ones_col, tag):
    """z_sb [64(c), 64(pq)] fp16 -> (q_ps, s_ps): q_ps [64(c), 64(pq)] fp32
    PSUM = (K@Vw/64).T @ exp(beta S) UNNORMALIZED, s_ps [1, 64] fp32 PSUM =
    softmax denominators. Scores in S^T layout [m(4x128), pq]; no transposes,
    no normalization here (callers fold 1/s in downstream)."""
    ST = ps.tile([128, 256], F32, tag="S", bufs=2, name=f"ST{tag}")
    for t in range(4):
        nc.tensor.matmul(ST[:, t * 64:(t + 1) * 64],
                         KT[:, t * 128:(t + 1) * 128], z_sb,
                         start=True, stop=True)
    att = sb.tile([128, 256], F16, tag=f"att{tag}", name=f"att{tag}")
    nc.scalar.activation(out=att[:], in_=ST[:], func=AF.Exp,
                         bias=0.0, scale=BETA)
    qs_ps = ps.tile([65, 64], F32, tag="q64", bufs=2, name=f"qs{tag}")
    if tag == "2":
        # hf2: q first -- s2 is only DMA'd out, q gates the output copy
        for t in range(4):
            nc.tensor.matmul(qs_ps[0:64, :], KV[:, t, :],
                             att[:, t * 64:(t + 1) * 64],
                             start=(t == 0), stop=(t == 3))
        for t in range(4):
            nc.tensor.matmul(qs_ps[64:65, :], ones_col,
                             att[:, t * 64:(t + 1) * 64],
                             start=(t == 0), stop=(t == 3))
    else:
        for t in range(4):
            nc.tensor.matmul(qs_ps[64:65, :], ones_col,
                             att[:, t * 64:(t + 1) * 64],
                             start=(t == 0), stop=(t == 3))
        for t in range(4):
            nc.tensor.matmul(qs_ps[0:64, :], KV[:, t, :],
                             att[:, t * 64:(t + 1) * 64],
                             start=(t == 0), stop=(t == 3))
    return qs_ps


def _build_nc(debug=False):
    nc = bacc.Bacc("TRN2", target_bir_lowering=False, debug=False,
                   num_devices=N_CORES)
    d_main = nc.dram_tensor("main", [48, 289], F16, kind="ExternalInput")
    d_cv2 = nc.dram_tensor("cv2", [64, 1025], F16, kind="ExternalInput")
    d_hop = nc.dram_tensor("hop", [128, 832], F16, kind="ExternalInput")
    d_wB = nc.dram_tensor("wB", [128, 2196], F16, kind="ExternalInput")
    d_wC = nc.dram_tensor("wC", [128, 1792], F16, kind="ExternalInput")
    d_smpl = nc.dram_tensor("smpl", [100, 192], F16, kind="ExternalInput")
    out_t = nc.dram_tensor("out", [65, 64], F32, kind="ExternalOutput")

    with tile.TileContext(nc) as tc:
        with tc.tile_pool(name="sb", bufs=1) as sb, \
             tc.tile_pool(name="ps", bufs=1, space="PSUM") as ps:
            # ---- PE warm-up ASAP: pe_busy_start anchors the p-state ramp;
            # full speed arrives 3us after the first PE instruction.
            warm = sb.tile([2, 8], F16, tag="warm")
            nc.gpsimd.memset(warm[:], 0.0)
            for w_ in range(3):
                warm_ps = ps.tile([8, 8], F32, tag="q64", bufs=2,
                                  name=f"warm{w_}")
                nc.tensor.matmul(warm_ps[:], warm[0:2, :], warm[0:2, :],
                                 start=True, stop=True)

            # ---- input DMAs, ordered by first use (HWDGE serializes)
            main = sb.tile([48, 289], F16, tag="main")
            nc.sync.dma_start(out=main[:], in_=d_main[:])
            cv2 = sb.tile([64, 1025], F16, tag="cv2")
            nc.scalar.dma_start(out=cv2[:], in_=d_cv2[:])
            hop = sb.tile([128, 832], F16, tag="hop")
            nc.sync.dma_start(out=hop[:], in_=d_hop[:])
            wB = sb.tile([128, 2196], F16, tag="wB")
            nc.scalar.dma_start(out=wB[:], in_=d_wB[:])
            smpl = sb.tile([100, 192], F16, tag="smpl")
            nc.sync.dma_start(out=smpl[:], in_=d_smpl[:])
            wC = sb.tile([128, 1792], F16, tag="wC")
            nc.scalar.dma_start(out=wC[:], in_=d_wC[:])

            P1 = main[0:48, 0:256]
            w1f = main[0:48, 256:288]
            b1_16 = main[0:32, 288:289]
            b2_16 = cv2[0:64, 1024:1025]
            w2ta = cv2[0:32, 0:1024].rearrange("c (i o) -> c i o", i=16)
            w2fT = wC[:, 1536:1792].rearrange("k (t c) -> k t c", t=4)
            KT = hop[0:64, 0:512]
            KV = hop[:, 576:832].rearrange("k (t c) -> k t c", t=4)
            w2b = wB[0:64, 0:512]
            Scomb = wB[:, 512:912].rearrange("k (t u) -> k t u", t=4)
            CandM = wB[0:100, 912:1296].rearrange("u (c k) -> u c k", c=3)
            PermB = wB[0:16, 1296:2196]
            W1big = wC[0:100, 0:1536].rearrange("u (h t k) -> u h t k",
                                                h=3, t=4)
            X = smpl[:].rearrange("u (h q) -> u h q", h=3)

            # ---- Pool: constants + zero-fills, all off the critical path
            ones_col = sb.tile([128, 1], F16, tag="ones_col")
            nc.gpsimd.memset(ones_col[:], 1.0)
            ones_row = sb.tile([1, 100], F16, tag="ones_row")
            nc.gpsimd.memset(ones_row[:], 1.0)
            a1p = sb.tile([32, 18, 18], F16, tag="a1p")
            nc.gpsimd.memset(a1p[:], 0.0)
            cstk = sb.tile([16, 8, 8, 9], F16, tag="cstk")
            nc.gpsimd.memset(cstk[:], 0.0)
            eB = sb.tile([16, 12, 8], F16, tag="eB")
            nc.gpsimd.memset(eB[:], 0.0)

            # ---- biases to fp32 (DVE tensor_scalar needs fp32 scalar APs)
            b1c = sb.tile([32, 1], F32, tag="b1c")
            nc.vector.tensor_copy(out=b1c[:], in_=b1_16)

            # ---- conv1 + relu into padded a1p [32, 18, 18]
            a1_ps = ps.tile([32, 256], F32, tag="a1", bufs=1)
            nc.tensor.matmul(a1_ps[:], w1f, P1, start=True, stop=True)
            nc.vector.tensor_scalar(
                out=a1p[:, 1:17, 1:17],
                in0=a1_ps[:].rearrange("c (p q) -> c p q", p=16),
                scalar1=b1c[:], scalar2=0.0, op0=ALU.add, op1=ALU.max)

            # ---- conv2 + relu directly from strided a1p windows:
            # rhs(t,a)[c1, p, q] = a1p[c1, 2p+t, 2q+a]
            a1p_ap = a1p[:]
            z2_ps = ps.tile([64, 64], F32, tag="q64", bufs=2)
            i = 0
            for t in range(4):
                for a in range(4):
                    rhs = bass.AP(
                        tensor=a1p_ap.tensor,
                        offset=a1p_ap.offset + t * 18 + a,
                        ap=[[324, 32], [36, 8], [2, 8]])
                    nc.tensor.matmul(
                        z2_ps[:], w2ta[:, t * 4 + a, :], rhs,
                        start=(i == 0), stop=(i == 15))
                    i += 1
            z2 = sb.tile([64, 64], F16, tag="z2")
            nc.scalar.activation(out=z2[:], in_=z2_ps[:], func=AF.Relu,
                                 bias=b2_16, scale=1.0)

            # ---- relu-derivative masks, off the critical path:
            # M1W[a*32+c1, t, pq] = (a1p[c1, 2p+t, 2q+a] != 0)
            M1W = sb.tile([128, 4, 64], F16, tag="M1W")
            for a in range(4):
                src = bass.AP(
                    tensor=a1p_ap.tensor,
                    offset=a1p_ap.offset + a,
                    ap=[[324, 32], [18, 4], [36, 8], [2, 8]])
                dst = M1W[a * 32:(a + 1) * 32, :, :].rearrange(
                    "k t (p q) -> k t p q", p=8)
                nc.vector.tensor_scalar(out=dst, in0=src, scalar1=0.0,
                                        scalar2=None, op0=ALU.not_equal)
            m2 = sb.tile([64, 64], F16, tag="m2")
            nc.vector.tensor_scalar(out=m2[:], in0=z2[:], scalar1=0.0,
                                    scalar2=None, op0=ALU.not_equal)

            # ---- Hopfield #1. Instead of normalizing q (1/s on the free
            # axis is awkward), scale z2 by s/64: Dm2m' = (z2*(s/64) - q)*m2
            # = (s/64)*Dm2m_true, a column scale that commutes through the
            # whole backward; 1/(s/64) is applied once at C16, off-chain.
            qs1 = _hopfield(nc, sb, ps, z2[:], KT, KV, ones_col[:], "1")
            s1row = sb.tile([1, 64], F16, tag="s1row")
            nc.vector.tensor_scalar_mul(s1row[:], qs1[64:65, :], 1.0 / 64.0)
            # qm2 = q*m2 runs while the s-broadcast round-trips through PE
            qm2 = sb.tile([64, 64], F16, tag="qm2")
            nc.vector.tensor_tensor(out=qm2[:], in0=qs1[0:64, :], in1=m2[:],
                                    op=ALU.mult)
            sb1_ps = ps.tile([64, 64], F32, tag="q64", bufs=2, name="sb1")
            nc.tensor.matmul(sb1_ps[:], ones_row[0:1, 0:64], s1row[:],
                             start=True, stop=True)
            z2s = sb.tile([64, 64], F16, tag="z2s")
            nc.vector.tensor_tensor(out=z2s[:], in0=z2[:], in1=sb1_ps[:],
                                    op=ALU.mult)
            # (z2*sb1 - q)*m2 == z2s - q*m2 because z2*m2 == z2
            Dm2m = sb.tile([64, 64], F16, tag="Dm2m")
            dm2m_inst = nc.vector.tensor_tensor(out=Dm2m[:], in0=z2s[:],
                                                in1=qm2[:], op=ALU.subtract)
            # off-chain (issued after Dm2m so they don't sit on the spine):
            # recB100[uv, pq] = 64/s[pq] for the C un-scaling
            r1row = sb.tile([1, 64], F16, tag="r1row")
            with nc.allow_low_precision(reason="softmax 1/sum in fp16"):
                recip_inst = nc.vector.reciprocal(r1row[:], s1row[:])
            add_dep_helper(dm2m_inst.ins, recip_inst.ins, sync=False,
                           reason="recip only feeds C16; keep Dm2m spine hot")
            rb100_ps = ps.tile([100, 64], F32, tag="g128", bufs=3,
                               name="rb100")
            nc.tensor.matmul(rb100_ps[:], ones_row[:], r1row[:],
                             start=True, stop=True)
            recB100 = sb.tile([100, 64], F16, tag="recB100")
            nc.scalar.copy(out=recB100[:], in_=rb100_ps[:])

            # ---- backward: g1m = (w2b^T @ Dm2m) * M1W, all 4 chunks in one
            # PSUM tile + one DVE multiply
            g1_ps = ps.tile([128, 256], F32, tag="S", bufs=2)
            for t in range(4):
                nc.tensor.matmul(g1_ps[:, t * 64:(t + 1) * 64],
                                 w2b[:, t * 128:(t + 1) * 128], Dm2m[:],
                                 start=True, stop=True)
            g1m = sb.tile([128, 4, 64], F16, tag="g1m")
            nc.vector.tensor_tensor(
                out=g1m[:].rearrange("k t u -> k (t u)"), in0=g1_ps[:],
                in1=M1W[:].rearrange("k t u -> k (t u)"), op=ALU.mult)

            # ---- C [100, 64] = sum_t Scomb_t^T @ g1m_t, then fp16 snapshot
            C_ps = ps.tile([100, 64], F32, tag="a1", bufs=1)
            for t in range(4):
                nc.tensor.matmul(C_ps[:], Scomb[:, t, :], g1m[:, t, :],
                                 start=(t == 0), stop=(t == 3))
            C16 = sb.tile([100, 64], F16, tag="C16")
            nc.vector.tensor_tensor(out=C16[:], in0=C_ps[:], in1=recB100[:],
                                    op=ALU.mult)

            # ---- blocked e_min: 3 candidate matmuls, shifted stack, min
            cand = [None] * 3
            for cc in range(3):
                cand[cc] = ps.tile([128, 8, 8], F32, tag="g128", bufs=3,
                                   name=f"cand{cc}")
                nc.tensor.matmul(
                    cand[cc][:].rearrange("k p q -> k (p q)"),
                    CandM[:, cc, :], C16[:], start=True, stop=True)
            # stage cand -> fp16 SBUF once; the 9 shifted stack copies are
            # then all-fp16-SBUF (4x DVE mode) and Pool can take a share
            candS = sb.tile([128, 3, 8, 8], F16, tag="candS")
            nc.vector.tensor_copy(out=candS[:, 0, :, :], in_=cand[0][:])
            nc.scalar.copy(out=candS[:, 1, :, :], in_=cand[1][:])
            nc.vector.tensor_copy(out=candS[:, 2, :, :], in_=cand[2][:])
            for j, k in enumerate([0, 1, 2, 3, 5, 6, 7, 8]):
                cc, kk = divmod(k, 4)
                dp, dq = k // 3 - 1, k % 3 - 1
                i4lo, i4hi = max(0, dp), min(8, 8 + dp)
                j4lo, j4hi = max(0, dq), min(8, 8 + dq)
                srcap = candS[kk * 32:kk * 32 + 16, cc,
                              i4lo - dp:i4hi - dp,
                              j4lo - dq:j4hi - dq, None]
                dstap = cstk[:, i4lo:i4hi, j4lo:j4hi, j:j + 1]
                if j in (1, 3):
                    nc.scalar.copy(out=dstap, in_=srcap)
                elif j in (5, 7):
                    nc.gpsimd.tensor_copy(out=dstap, in_=srcap)
                else:
                    nc.vector.tensor_copy(out=dstap, in_=srcap)
            nc.vector.tensor_copy(out=cstk[:, :, :, 8:9],
                                  in_=candS[0:16, 1, :, :, None])
            nc.vector.tensor_reduce(out=eB[:, 2:10, :], in_=cstk[:],
                                    axis=mybir.AxisListType.X, op=ALU.min)

            # ---- eW gather (one-hot PermB) + mask
            eBf = eB[:].rearrange("a b c -> a (b c)")
            eW_ps = ps.tile([100, 64], F32, tag="a1", bufs=1)
            for k in range(9):
                dp, dq = k // 3 - 1, k % 3 - 1
                off = 16 + 8 * dp + dq
                nc.tensor.matmul(eW_ps[:], PermB[:, k * 100:(k + 1) * 100],
                                 eBf[:, off:off + 64],
                                 start=(k == 0), stop=(k == 8))
            maskw = sb.tile([100, 64], F16, tag="maskw")
            nc.vector.tensor_tensor(out=maskw[:], in0=C16[:], in1=eW_ps[:],
                                    op=ALU.is_le)

            # ---- masked forward: Xm = X * maskw (broadcast over h)
            Xm = sb.tile([100, 3, 64], F16, tag="Xm")
            mask_b = bass.AP(tensor=maskw[:].tensor, offset=maskw[:].offset,
                             ap=[[64, 100], [0, 3], [1, 64]])
            nc.vector.tensor_tensor(out=Xm[:], in0=X, in1=mask_b, op=ALU.mult)
            u1_ps = ps.tile([128, 256], F32, tag="S", bufs=2)
            for t in range(4):
                for h in range(3):
                    nc.tensor.matmul(u1_ps[:, t * 64:(t + 1) * 64],
                                     W1big[:, h, t, :], Xm[:, h, :],
                                     start=(h == 0), stop=(h == 2))
            u1m = sb.tile([128, 4, 64], F16, tag="u1m")
            nc.vector.tensor_tensor(
                out=u1m[:].rearrange("k t u -> k (t u)"), in0=u1_ps[:],
                in1=M1W[:].rearrange("k t u -> k (t u)"), op=ALU.mult)
            zm_ps = ps.tile([64, 64], F32, tag="q64", bufs=2)
            for t in range(4):
                nc.tensor.matmul(zm_ps[:], w2fT[:, t, :], u1m[:, t, :],
                                 start=(t == 0), stop=(t == 3))
            z2m = sb.tile([64, 64], F16, tag="z2m")
            nc.vector.tensor_tensor(out=z2m[:], in0=zm_ps[:], in1=m2[:],
                                    op=ALU.mult)

            # ---- Hopfield #2 -> ship q2/64 (rows 0:64) and s2 (row 64);
            # the host computes out = 64*q2s/s2.
            qs2 = _hopfield(nc, sb, ps, z2m[:], KT, KV, ones_col[:], "2")
            out_sb = sb.tile([65, 64], F32, tag="out_sb")
            nc.vector.tensor_copy(out=out_sb[:], in_=qs2[:])
            nc.sync.dma_start(out=out_t[:], in_=out_sb[:])
    nc.compile()
    return nc


def _get_nc(debug=False):
    key = ("nc", debug)
    if key not in _CACHE:
        _CACHE[key] = _build_nc(debug)
    return _CACHE[key]


# ---------------------------------------------------------------- entry point
def kernel(x, w1, b1, w2, b2, K, Vw, _debug=False):
    x = np.asarray(x, np.float32)
    shared = _host_prep(np.asarray(w1, np.float32), np.asarray(b1, np.float32),
                        np.asarray(w2, np.float32), np.asarray(b2, np.float32),
                        np.asarray(K, np.float32), np.asarray(Vw, np.float32))
    bsz = x.shape[0]
    nc = _get_nc(False)
    smpls = [_sample_prep(x[b]) for b in range(bsz)]
    in_maps = []
    for core in range(N_CORES):
        P1b, Xb = smpls[core] if core < bsz else smpls[0]
        mainb = shared["main"].copy()
        mainb[0:48, 0:256] = P1b
        m = {"main": mainb, "cv2": shared["cv2"], "hop": shared["hop"],
             "wB": shared["wB"], "wC": shared["wC"], "smpl": Xb}
        in_maps.append(m)
    res = run_bass_kernel_spmd(nc, in_maps, core_ids=list(range(N_CORES)))
    outs = []
    for b in range(bsz):
        r = np.asarray(res.results[b]["out"], np.float32)
        outs.append((64.0 * r[0:64] / r[64:65]).reshape(64, 8, 8))
    out = np.stack(outs).astype(np.float32)
    if _debug:
        return out, res
    return out


# revision 29
# speedup vs baseline: 1.0096x; 1.0043x over previous
"""TRN2 Bass kernel for nn_Block1_43542378447225 (fp16 rewrite).

Per sample on one NeuronCore (batch=2 -> cores 0/1 do real work):
  conv1 -> relu into padded a1p -> conv2 directly from strided a1p windows
  -> z2 -> Hopfield #1 in S^T layout -> Dm2 -> backward (w2b, Scomb) -> C
  -> blocked e_min (shifted candidate stack + min-reduce) -> eW gather
  -> mask -> masked patch forward (W1big) -> z2_masked -> Hopfield #2 -> out

All SBUF data fp16 (PE 4x faster than fp32, DVE 2x); PSUM fp32.
The C -> e_min -> mask comparison path stays bit-exact in fp16: cand/eW
matmuls are one-hot gathers, so every candidate equals an fp16-rounded C
entry exactly and the argmin survives `C16 <= eW`.

Hopfield is computed transpose-free: scores S^T[m, pq] = (K_chunk @ z) via
4 matmuls, exp on ACT, softmax denominators via an ones-column matmul.
Softmax 1/s is never applied to q on the spine: for Hopfield #1 the update
direction uses Dm2m' = (z2*(s/64) - q*m2), an s/64 column scale that
commutes through the backward and is cancelled once at C16 (rank-1
broadcast of 64/s); Hopfield #2 ships unnormalized q2 plus s2 and the
host divides.

Layout: pq = p*8+q (64 conv2 output positions), uv = u*10+v (100 composite
window offsets), kc = a*32 + c1 (hidden index; chunk t = conv2 kernel row,
a = conv2 kernel col).
"""
import numpy as np

import concourse.bass as bass
import concourse.bacc as bacc
import concourse.mybir as mybir
import concourse.tile as tile
from concourse.tile import add_dep_helper
from concourse.bass_utils import run_bass_kernel_spmd

F32 = mybir.dt.float32
F16 = mybir.dt.float16
AF = mybir.ActivationFunctionType
ALU = mybir.AluOpType

N_CORES = 8
BETA = 0.125  # 1/sqrt(64)

_CACHE = {}


# ---------------------------------------------------------------- host prep
def _build_scomb_w1big(w1):
    w1s = w1.sum(axis=1)
    Scomb = np.zeros((4, 32, 4, 100), np.float32)  # [a, c1, t, uv]
    W1big = np.zeros((100, 3, 4, 4, 32), np.float32)  # [uv, h, t, a, c1]
    for t in range(4):
        for a in range(4):
            for u in range(10):
                ki = u - 2 * t
                if not (0 <= ki < 4):
                    continue
                for v in range(10):
                    kj = v - 2 * a
                    if not (0 <= kj < 4):
                        continue
                    Scomb[a, :, t, u * 10 + v] = w1s[:, ki, kj]
                    W1big[u * 10 + v, :, t, a, :] = w1[:, :, ki, kj].T
    # partition index = a*32+c1 -> merge (a, c1); free = t*100+uv
    return Scomb.reshape(128, 400), W1big.reshape(100, 1536)


def _host_prep(w1, b1, w2, b2, K, Vw):
    # cv1 template [48, 289]: per-sample P1 (cols 0:256) filled later;
    # w1f | b1 shared. Small first DMA -> conv1 starts earliest.
    main = np.zeros((48, 289), np.float16)
    main[:, 256:288] = np.transpose(w1, (2, 3, 1, 0)).reshape(48, 32)
    main[0:32, 288:289] = b1[:, None]

    # conv2 weights with every (t, a) block based at partition 0:
    # cv2[c1, (t*4+a)*64 + o] = w2[o, c1, t, a]; b2 in col 1024
    cv2 = np.zeros((64, 1025), np.float16)
    cv2[0:32, 0:1024] = np.transpose(w2, (1, 2, 3, 0)).reshape(32, 1024)
    cv2[0:64, 1024:1025] = b2[:, None]

    hop = np.zeros((128, 832), np.float16)
    hop[0:64, 0:512] = K.T
    hop[0:64, 512:576] = Vw
    # KV chunks [128, 4, 64]: KV[m, e] = (K @ Vw)[m, e] / 64 (the 1/64 keeps
    # the s/64-scaled backward inside fp16 range; host divides it back out)
    KVh = (K @ Vw).astype(np.float32).reshape(4, 128, 64) / 64.0
    hop[:, 576:832] = np.transpose(KVh, (1, 0, 2)).reshape(128, 256)

    Scomb, W1big = _build_scomb_w1big(w1)
    PermF = np.zeros((100, 9, 16), np.float32)
    for k in range(9):
        dp, dq = k // 3 - 1, k % 3 - 1
        for im in range(4):
            u = 4 * dp + im + 3
            if not (0 <= u < 10):
                continue
            for jm in range(4):
                v = 4 * dq + jm + 3
                if not (0 <= v < 10):
                    continue
                PermF[u * 10 + v, k, im * 4 + jm] = 1.0
    CandM = np.zeros((100, 3, 128), np.float32)
    for k in range(9):
        cc, kk = divmod(k, 4)
        CandM[:, cc, kk * 32:kk * 32 + 16] = PermF[:, k, :]
    PermB = np.transpose(PermF, (2, 1, 0)).reshape(16, 900)

    # wB [128, 2196]: w2b (2.0 folded) | Scomb | CandM | PermB
    wB = np.zeros((128, 2196), np.float16)
    wB[0:64, 0:512] = 2.0 * np.transpose(w2, (0, 2, 3, 1)).reshape(64, 512)
    wB[:, 512:912] = Scomb
    wB[0:100, 912:1296] = CandM.reshape(100, 384)
    wB[0:16, 1296:2196] = PermB

    wC = np.zeros((128, 1792), np.float16)
    wC[0:100, 0:1536] = W1big
    wC[:, 1536:1792] = np.transpose(w2, (3, 1, 2, 0)).reshape(128, 256)
    return {"main": main, "cv2": cv2, "hop": hop, "wB": wB, "wC": wC}


def _sample_prep(x_s):
    xp1 = np.pad(x_s, ((0, 0), (1, 1), (1, 1)))
    xp3 = np.pad(x_s, ((0, 0), (3, 3), (3, 3)))
    P1 = np.zeros((4, 4, 3, 16, 16), np.float32)
    for kr in range(4):
        for ks in range(4):
            P1[kr, ks] = xp1[:, kr:kr + 32:2, ks:ks + 32:2][:, :16, :16]
    X = np.zeros((10, 10, 3, 8, 8), np.float32)
    for u in range(10):
        for v in range(10):
            X[u, v] = xp3[:, u:u + 32:4, v:v + 32:4][:, :8, :8]
    return (P1.reshape(48, 256).astype(np.float16),
            X.reshape(100, 192).astype(np.float16))


# ---------------------------------------------------------------- device build
def _hopfield(nc, sb, ps, z_sb, KT, KV, ones_col, tag):
    """z_sb [64(c), 64(pq)] fp16 -> (q_ps, s_ps): q_ps [64(c), 64(pq)] fp32
    PSUM = (K@Vw/64).T @ exp(beta S) UNNORMALIZED, s_ps [1, 64] fp32 PSUM =
    softmax denominators. Scores in S^T layout [m(4x128), pq]; no transposes,
    no normalization here (callers fold 1/s in downstream)."""
    ST = ps.tile([128, 256], F32, tag="S", bufs=2, name=f"ST{tag}")
    for t in range(4):
        nc.tensor.matmul(ST[:, t * 64:(t + 1) * 64],
                         KT[:, t * 128:(t + 1) * 128], z_sb,
                         start=True, stop=True)
    att = sb.tile([128, 256], F16, tag=f"att{tag}", name=f"att{tag}")
    nc.scalar.activation(out=att[:], in_=ST[:], func=AF.Exp,
                         bias=0.0, scale=BETA)
    if tag == "2":
        # hf2: q in cols 0:64, s as a free-size-1 column matmul in col 64
        # (att chunk as lhsT, ones as rhs -> per-partition sums, ~free on PE)
        qs_ps = ps.tile([64, 65], F32, tag="q64", bufs=2, name=f"qs{tag}")
        for t in range(4):
            nc.tensor.matmul(qs_ps[:, 0:64], KV[:, t, :],
                             att[:, t * 64:(t + 1) * 64],
                             start=(t == 0), stop=(t == 3))
        for t in range(4):
            nc.tensor.matmul(qs_ps[:, 64:65], att[:, t * 64:(t + 1) * 64],
                             ones_col, start=(t == 0), stop=(t == 3))
        return qs_ps
    qs_ps = ps.tile([65, 64], F32, tag="q64", bufs=2, name=f"qs{tag}")
    for t in range(4):
        nc.tensor.matmul(qs_ps[64:65, :], ones_col,
                         att[:, t * 64:(t + 1) * 64],
                         start=(t == 0), stop=(t == 3))
    for t in range(4):
        nc.tensor.matmul(qs_ps[0:64, :], KV[:, t, :],
                         att[:, t * 64:(t + 1) * 64],
                         start=(t == 0), stop=(t == 3))
    return qs_ps


def _build_nc(debug=False):
    nc = bacc.Bacc("TRN2", target_bir_lowering=False, debug=False,
                   num_devices=N_CORES)
    d_main = nc.dram_tensor("main", [48, 289], F16, kind="ExternalInput")
    d_cv2 = nc.dram_tensor("cv2", [64, 1025], F16, kind="ExternalInput")
    d_hop = nc.dram_tensor("hop", [128, 832], F16, kind="ExternalInput")
    d_wB = nc.dram_tensor("wB", [128, 2196], F16, kind="ExternalInput")
    d_wC = nc.dram_tensor("wC", [128, 1792], F16, kind="ExternalInput")
    d_smpl = nc.dram_tensor("smpl", [100, 192], F16, kind="ExternalInput")
    out_t = nc.dram_tensor("out", [64, 65], F32, kind="ExternalOutput")

    with tile.TileContext(nc) as tc:
        with tc.tile_pool(name="sb", bufs=1) as sb, \
             tc.tile_pool(name="ps", bufs=1, space="PSUM") as ps:
            # ---- PE warm-up ASAP: pe_busy_start anchors the p-state ramp;
            # full speed arrives 3us after the first PE instruction.
            warm = sb.tile([2, 8], F16, tag="warm")
            nc.gpsimd.memset(warm[:], 0.0)
            for w_ in range(3):
                warm_ps = ps.tile([8, 8], F32, tag="q64", bufs=2,
                                  name=f"warm{w_}")
                nc.tensor.matmul(warm_ps[:], warm[0:2, :], warm[0:2, :],
                                 start=True, stop=True)

            # ---- input DMAs, ordered by first use (HWDGE serializes)
            main = sb.tile([48, 289], F16, tag="main")
            nc.sync.dma_start(out=main[:], in_=d_main[:])
            cv2 = sb.tile([64, 1025], F16, tag="cv2")
            nc.scalar.dma_start(out=cv2[:], in_=d_cv2[:])
            hop = sb.tile([128, 832], F16, tag="hop")
            nc.sync.dma_start(out=hop[:], in_=d_hop[:])
            wB = sb.tile([128, 2196], F16, tag="wB")
            nc.scalar.dma_start(out=wB[:], in_=d_wB[:])
            smpl = sb.tile([100, 192], F16, tag="smpl")
            nc.sync.dma_start(out=smpl[:], in_=d_smpl[:])
            wC = sb.tile([128, 1792], F16, tag="wC")
            nc.scalar.dma_start(out=wC[:], in_=d_wC[:])

            P1 = main[0:48, 0:256]
            w1f = main[0:48, 256:288]
            b1_16 = main[0:32, 288:289]
            b2_16 = cv2[0:64, 1024:1025]
            w2ta = cv2[0:32, 0:1024].rearrange("c (i o) -> c i o", i=16)
            w2fT = wC[:, 1536:1792].rearrange("k (t c) -> k t c", t=4)
            KT = hop[0:64, 0:512]
            KV = hop[:, 576:832].rearrange("k (t c) -> k t c", t=4)
            w2b = wB[0:64, 0:512]
            Scomb = wB[:, 512:912].rearrange("k (t u) -> k t u", t=4)
            CandM = wB[0:100, 912:1296].rearrange("u (c k) -> u c k", c=3)
            PermB = wB[0:16, 1296:2196]
            W1big = wC[0:100, 0:1536].rearrange("u (h t k) -> u h t k",
                                                h=3, t=4)
            X = smpl[:].rearrange("u (h q) -> u h q", h=3)

            # ---- Pool: constants + zero-fills, all off the critical path
            ones_col = sb.tile([128, 1], F16, tag="ones_col")
            nc.gpsimd.memset(ones_col[:], 1.0)
            ones_row = sb.tile([1, 100], F16, tag="ones_row")
            nc.gpsimd.memset(ones_row[:], 1.0)
            a1p = sb.tile([32, 18, 18], F16, tag="a1p")
            nc.gpsimd.memset(a1p[:], 0.0)
            cstk = sb.tile([16, 8, 8, 9], F16, tag="cstk")
            nc.gpsimd.memset(cstk[:], 0.0)
            eB = sb.tile([16, 12, 8], F16, tag="eB")
            nc.gpsimd.memset(eB[:], 0.0)

            # ---- biases to fp32 (DVE tensor_scalar needs fp32 scalar APs)
            b1c = sb.tile([32, 1], F32, tag="b1c")
            nc.vector.tensor_copy(out=b1c[:], in_=b1_16)

            # ---- conv1 + relu into padded a1p [32, 18, 18]
            a1_ps = ps.tile([32, 256], F32, tag="a1", bufs=1)
            nc.tensor.matmul(a1_ps[:], w1f, P1, start=True, stop=True)
            nc.vector.tensor_scalar(
                out=a1p[:, 1:17, 1:17],
                in0=a1_ps[:].rearrange("c (p q) -> c p q", p=16),
                scalar1=b1c[:], scalar2=0.0, op0=ALU.add, op1=ALU.max)

            # ---- conv2 + relu directly from strided a1p windows:
            # rhs(t,a)[c1, p, q] = a1p[c1, 2p+t, 2q+a]
            a1p_ap = a1p[:]
            z2_ps = ps.tile([64, 64], F32, tag="q64", bufs=2)
            i = 0
            for t in range(4):
                for a in range(4):
                    rhs = bass.AP(
                        tensor=a1p_ap.tensor,
                        offset=a1p_ap.offset + t * 18 + a,
                        ap=[[324, 32], [36, 8], [2, 8]])
                    nc.tensor.matmul(
                        z2_ps[:], w2ta[:, t * 4 + a, :], rhs,
                        start=(i == 0), stop=(i == 15))
                    i += 1
            z2 = sb.tile([64, 64], F16, tag="z2")
            nc.scalar.activation(out=z2[:], in_=z2_ps[:], func=AF.Relu,
                                 bias=b2_16, scale=1.0)

            # ---- relu-derivative masks, off the critical path:
            # M1W[a*32+c1, t, pq] = (a1p[c1, 2p+t, 2q+a] != 0)
            M1W = sb.tile([128, 4, 64], F16, tag="M1W")
            for a in range(4):
                src = bass.AP(
                    tensor=a1p_ap.tensor,
                    offset=a1p_ap.offset + a,
                    ap=[[324, 32], [18, 4], [36, 8], [2, 8]])
                dst = M1W[a * 32:(a + 1) * 32, :, :].rearrange(
                    "k t (p q) -> k t p q", p=8)
                nc.vector.tensor_scalar(out=dst, in0=src, scalar1=0.0,
                                        scalar2=None, op0=ALU.not_equal)
            m2 = sb.tile([64, 64], F16, tag="m2")
            nc.vector.tensor_scalar(out=m2[:], in0=z2[:], scalar1=0.0,
                                    scalar2=None, op0=ALU.not_equal)

            # ---- Hopfield #1. Instead of normalizing q (1/s on the free
            # axis is awkward), scale z2 by s/64: Dm2m' = (z2*(s/64) - q)*m2
            # = (s/64)*Dm2m_true, a column scale that commutes through the
            # whole backward; 1/(s/64) is applied once at C16, off-chain.
            qs1 = _hopfield(nc, sb, ps, z2[:], KT, KV, ones_col[:], "1")
            s1row = sb.tile([1, 64], F16, tag="s1row")
            nc.vector.tensor_scalar_mul(s1row[:], qs1[64:65, :], 1.0 / 64.0)
            # qm2 = q*m2 runs while the s-broadcast round-trips through PE
            qm2 = sb.tile([64, 64], F16, tag="qm2")
            nc.vector.tensor_tensor(out=qm2[:], in0=qs1[0:64, :], in1=m2[:],
                                    op=ALU.mult)
            sb1_ps = ps.tile([64, 64], F32, tag="q64", bufs=2, name="sb1")
            nc.tensor.matmul(sb1_ps[:], ones_row[0:1, 0:64], s1row[:],
                             start=True, stop=True)
            z2s = sb.tile([64, 64], F16, tag="z2s")
            nc.vector.tensor_tensor(out=z2s[:], in0=z2[:], in1=sb1_ps[:],
                                    op=ALU.mult)
            # (z2*sb1 - q)*m2 == z2s - q*m2 because z2*m2 == z2
            Dm2m = sb.tile([64, 64], F16, tag="Dm2m")
            dm2m_inst = nc.vector.tensor_tensor(out=Dm2m[:], in0=z2s[:],
                                                in1=qm2[:], op=ALU.subtract)
            # off-chain (issued after Dm2m so they don't sit on the spine):
            # recB100[uv, pq] = 64/s[pq] for the C un-scaling
            r1row = sb.tile([1, 64], F16, tag="r1row")
            with nc.allow_low_precision(reason="softmax 1/sum in fp16"):
                recip_inst = nc.vector.reciprocal(r1row[:], s1row[:])
            add_dep_helper(dm2m_inst.ins, recip_inst.ins, sync=False,
                           reason="recip only feeds C16; keep Dm2m spine hot")
            rb100_ps = ps.tile([100, 64], F32, tag="g128", bufs=3,
                               name="rb100")
            nc.tensor.matmul(rb100_ps[:], ones_row[:], r1row[:],
                             start=True, stop=True)
            recB100 = sb.tile([100, 64], F16, tag="recB100")
            nc.scalar.copy(out=recB100[:], in_=rb100_ps[:])

            # ---- backward: g1m = (w2b^T @ Dm2m) * M1W, all 4 chunks in one
            # PSUM tile + one DVE multiply
            g1_ps = ps.tile([128, 256], F32, tag="S", bufs=2)
            for t in range(4):
                nc.tensor.matmul(g1_ps[:, t * 64:(t + 1) * 64],
                                 w2b[:, t * 128:(t + 1) * 128], Dm2m[:],
                                 start=True, stop=True)
            g1m = sb.tile([128, 4, 64], F16, tag="g1m")
            nc.vector.tensor_tensor(
                out=g1m[:].rearrange("k t u -> k (t u)"), in0=g1_ps[:],
                in1=M1W[:].rearrange("k t u -> k (t u)"), op=ALU.mult)

            # ---- C [100, 64] = sum_t Scomb_t^T @ g1m_t, then fp16 snapshot
            C_ps = ps.tile([100, 64], F32, tag="a1", bufs=1)
            for t in range(4):
                nc.tensor.matmul(C_ps[:], Scomb[:, t, :], g1m[:, t, :],
                                 start=(t == 0), stop=(t == 3))
            C16 = sb.tile([100, 64], F16, tag="C16")
            nc.vector.tensor_tensor(out=C16[:], in0=C_ps[:], in1=recB100[:],
                                    op=ALU.mult)

            # ---- blocked e_min: 3 candidate matmuls, shifted stack, min
            cand = [None] * 3
            for cc in range(3):
                cand[cc] = ps.tile([128, 8, 8], F32, tag="g128", bufs=3,
                                   name=f"cand{cc}")
                nc.tensor.matmul(
                    cand[cc][:].rearrange("k p q -> k (p q)"),
                    CandM[:, cc, :], C16[:], start=True, stop=True)
            # stage cand -> fp16 SBUF once; the 9 shifted stack copies are
            # then all-fp16-SBUF (4x DVE mode) and Pool can take a share
            candS = sb.tile([128, 3, 8, 8], F16, tag="candS")
            nc.vector.tensor_copy(out=candS[:, 0, :, :], in_=cand[0][:])
            nc.scalar.copy(out=candS[:, 1, :, :], in_=cand[1][:])
            nc.vector.tensor_copy(out=candS[:, 2, :, :], in_=cand[2][:])
            for j, k in enumerate([0, 1, 2, 3, 5, 6, 7, 8]):
                cc, kk = divmod(k, 4)
                dp, dq = k // 3 - 1, k % 3 - 1
                i4lo, i4hi = max(0, dp), min(8, 8 + dp)
                j4lo, j4hi = max(0, dq), min(8, 8 + dq)
                srcap = candS[kk * 32:kk * 32 + 16, cc,
                              i4lo - dp:i4hi - dp,
                              j4lo - dq:j4hi - dq, None]
                dstap = cstk[:, i4lo:i4hi, j4lo:j4hi, j:j + 1]
                if j in (1, 3):
                    nc.scalar.copy(out=dstap, in_=srcap)
                elif j in (5, 7):
                    nc.gpsimd.tensor_copy(out=dstap, in_=srcap)
                else:
                    nc.vector.tensor_copy(out=dstap, in_=srcap)
            nc.vector.tensor_copy(out=cstk[:, :, :, 8:9],
                                  in_=candS[0:16, 1, :, :, None])
            nc.vector.tensor_reduce(out=eB[:, 2:10, :], in_=cstk[:],
                                    axis=mybir.AxisListType.X, op=ALU.min)

            # ---- eW gather (one-hot PermB) + mask
            eBf = eB[:].rearrange("a b c -> a (b c)")
            eW_ps = ps.tile([100, 64], F32, tag="a1", bufs=1)
            for k in range(9):
                dp, dq = k // 3 - 1, k % 3 - 1
                off = 16 + 8 * dp + dq
                nc.tensor.matmul(eW_ps[:], PermB[:, k * 100:(k + 1) * 100],
                                 eBf[:, off:off + 64],
                                 start=(k == 0), stop=(k == 8))
            maskw = sb.tile([100, 64], F16, tag="maskw")
            nc.vector.tensor_tensor(out=maskw[:], in0=C16[:], in1=eW_ps[:],
                                    op=ALU.is_le)

            # ---- masked forward: Xm = X * maskw (broadcast over h)
            Xm = sb.tile([100, 3, 64], F16, tag="Xm")
            mask_b = bass.AP(tensor=maskw[:].tensor, offset=maskw[:].offset,
                             ap=[[64, 100], [0, 3], [1, 64]])
            nc.vector.tensor_tensor(out=Xm[:], in0=X, in1=mask_b, op=ALU.mult)
            u1_ps = ps.tile([128, 256], F32, tag="S", bufs=2)
            for t in range(4):
                for h in range(3):
                    nc.tensor.matmul(u1_ps[:, t * 64:(t + 1) * 64],
                                     W1big[:, h, t, :], Xm[:, h, :],
                                     start=(h == 0), stop=(h == 2))
            u1m = sb.tile([128, 4, 64], F16, tag="u1m")
            nc.vector.tensor_tensor(
                out=u1m[:].rearrange("k t u -> k (t u)"), in0=u1_ps[:],
                in1=M1W[:].rearrange("k t u -> k (t u)"), op=ALU.mult)
            zm_ps = ps.tile([64, 64], F32, tag="q64", bufs=2)
            for t in range(4):
                nc.tensor.matmul(zm_ps[:], w2fT[:, t, :], u1m[:, t, :],
                                 start=(t == 0), stop=(t == 3))
            z2m = sb.tile([64, 64], F16, tag="z2m")
            nc.vector.tensor_tensor(out=z2m[:], in0=zm_ps[:], in1=m2[:],
                                    op=ALU.mult)

            # ---- Hopfield #2 -> ship q2/64 (rows 0:64) and s2 (row 64);
            # the host computes out = 64*q2s/s2.
            qs2 = _hopfield(nc, sb, ps, z2m[:], KT, KV, ones_col[:], "2")
            out_sb = sb.tile([64, 65], F32, tag="out_sb")
            nc.vector.tensor_copy(out=out_sb[:], in_=qs2[:])
            nc.sync.dma_start(out=out_t[:], in_=out_sb[:])
    nc.compile()
    return nc


def _get_nc(debug=False):
    key = ("nc", debug)
    if key not in _CACHE:
        _CACHE[key] = _build_nc(debug)
    return _CACHE[key]


# ---------------------------------------------------------------- entry point
def kernel(x, w1, b1, w2, b2, K, Vw, _debug=False):
    x = np.asarray(x, np.float32)
    shared = _host_prep(np.asarray(w1, np.float32), np.asarray(b1, np.float32),
                        np.asarray(w2, np.float32), np.asarray(b2, np.float32),
                        np.asarray(K, np.float32), np.asarray(Vw, np.float32))
    bsz = x.shape[0]
    nc = _get_nc(False)
    smpls = [_sample_prep(x[b]) for b in range(bsz)]
    in_maps = []
    for core in range(N_CORES):
        P1b, Xb = smpls[core] if core < bsz else smpls[0]
        mainb = shared["main"].copy()
        mainb[0:48, 0:256] = P1b
        m = {"main": mainb, "cv2": shared["cv2"], "hop": shared["hop"],
             "wB": shared["wB"], "wC": shared["wC"], "smpl": Xb}
        in_maps.append(m)
    res = run_bass_kernel_spmd(nc, in_maps, core_ids=list(range(N_CORES)))
    outs = []
    for b in range(bsz):
        r = np.asarray(res.results[b]["out"], np.float32)
        outs.append((64.0 * r[:, 0:64] / r[:, 64:65].T).reshape(64, 8, 8))
    out = np.stack(outs).astype(np.float32)
    if _debug:
        return out, res
    return out
